# revision 40
# baseline (speedup 1.0000x reference)
"""AMS loss kernel for Trainium2, data-parallel over 8 NeuronCores.

Reference computation (per row r of logits [N, C], target t_r):
    num_r   = logits[r, t_r]
    denom_r = exp(num_r) + (sum_j exp(logits[r, j])) * e^M - exp(num_r) * e^M
    L_r     = num_r - log(denom_r + EPS)
    out     = -mean_r(L_r)

Sharding: rows (N=16384) split evenly across 8 cores (2048 rows each).
Per core:
 - The target logits num_r are fetched straight from DRAM by 16 small
   indirect (gathering) DMAs on gpsimd's software DGE ([128, 1] each;
   the DGE takes one offset per partition per transfer), using
   host-packed flat element offsets -- no compute-engine gather pass.
 - The scalar engine streams the shard (16 row-tiles of [128, 10000])
   computing exp(x + M) with a fused per-row accumulate (accum_out).
 - The vector engine computes the tiny [128, 16] epilogue; gpsimd folds
   the per-row losses across partitions; the host sums 8 partial scalars.

Raw Bass (no Tile): Tile's auto-generated per-instruction waits overflow
the small sync-wait slot budgets of the fused-reduce and DMA instruction
formats, so synchronization is explicit standalone wait_ge per engine.

Schedule notes (from NTFF profiling):
 - The logits stream runs at HBM line rate (~425 GB/s best) on the SP
   HWDGE FIFO queue with 40 KB per-partition lines; the final tiles are
   column-split (CHW) so the last exposed exp after the stream ends is
   small, sized so exp service < chunk arrival in both clock regimes.
 - The chip power-shares between clock domains run-to-run (some runs
   have ~20% slower compute clocks, some ~15-20% slower HBM); with the
   gather off the vector engine, the scalar engine's exp is the only
   per-tile compute and it keeps slack in all regimes.
 - The epilogue ships partial [128, 1] per core; the host sums 1024
   floats and scales by -1/N.
"""

import sys
import numpy as np

for _p in ("/opt/trn_rl_repo",):
    if _p not in sys.path:
        sys.path.insert(0, _p)

N_TOTAL = 16384
C = 10000
N_CORES = 8
ROWS = N_TOTAL // N_CORES        # 2048 rows per core
P = 128                          # partitions
TILES = ROWS // P                # 16 row-tiles per core
M = 0.4
EPS = 1e-10
NBUF = 4                         # row-tile buffer slots

# chunk widths per tile: the last tiles are split (tapered) so the final
# exposed exp after the DMA stream ends is small
CHW = {12: [5000, 5000], 13: [5000, 5000], 14: [5000, 5000],
       15: [3400, 3400, 3200]}
CHN = [len(CHW.get(j, [0])) if j in CHW else 1 for j in range(TILES)]

PROFILE = False                  # set True (e.g. by test.py) to capture NTFF profile
LAST_RESULT = None               # BassKernelResults of the last run (for profiling)

_CACHE = {}


def _build_nc():
    from contextlib import ExitStack

    import concourse.bass as bass
    import concourse.mybir as mybir

    F32 = mybir.dt.float32
    FP8 = mybir.dt.float8e4
    I32 = mybir.dt.int32
    Alu = mybir.AluOpType
    Act = mybir.ActivationFunctionType

    EXP_M = float(np.exp(np.float32(M)))

    # chunk table: (tile j, chunk c, col_lo, col_hi)
    chunks = []
    for j in range(TILES):
        widths = CHW.get(j, [C])
        lo = 0
        for c, w in enumerate(widths):
            chunks.append((j, c, lo, lo + w))
            lo += w
        assert lo == C

    acum = [0] * (TILES + 1)     # cumulative exp count through tile j
    for j in range(TILES):
        acum[j + 1] = acum[j] + CHN[j]

    # multi-chunk tiles accumulate into scratch columns, folded at the end
    xcol = {}
    nx = 0
    for j in range(TILES):
        if CHN[j] > 1:
            for c in range(CHN[j]):
                xcol[(j, c)] = nx
                nx += 1
    N_FOLD = sum(1 for j in range(TILES) if CHN[j] > 1)

    A_E = acum[TILES]            # all exps done
    A_LND = A_E + 1
    V_FOLD = N_FOLD              # summ folds done
    V_DEN = V_FOLD + 1
    V_LG = V_DEN + 1

    slot_chunks = [0] * NBUF
    for j in range(TILES):
        slot_chunks[j % NBUF] = max(slot_chunks[j % NBUF], CHN[j])

    nc = bass.Bass()
    logits = nc.declare_dram_parameter("logits", [ROWS, C], F32, isOutput=False)
    toff = nc.declare_dram_parameter("toff", [P, TILES], I32, isOutput=False)
    out = nc.declare_dram_parameter("out", [P, 1], F32, isOutput=True)

    logits_t = logits.rearrange("(n p) c -> n p c", p=P)
    logits_flat = logits.rearrange("r c -> (r c) ()")

    with ExitStack() as ctx:
        en_ctx = ctx.enter_context
        tb = [
            en_ctx(nc.sbuf_tensor(f"tb{i}", [P, C], F32)) for i in range(NBUF)
        ]
        g_act = en_ctx(nc.sbuf_tensor([P, C], FP8))   # unused elementwise out
        bias_m = en_ctx(nc.sbuf_tensor([P, 1], F32))
        bias_eps = en_ctx(nc.sbuf_tensor([P, 1], F32))
        off_sb = en_ctx(nc.sbuf_tensor([P, TILES], I32))
        summ = en_ctx(nc.sbuf_tensor([P, TILES], F32))
        summ_x = en_ctx(nc.sbuf_tensor([P, max(nx, 1)], F32))
        num = en_ctx(nc.sbuf_tensor([P, TILES], F32))
        en = en_ctx(nc.sbuf_tensor([P, TILES], F32))
        denom = en_ctx(nc.sbuf_tensor([P, TILES], F32))
        lnd = en_ctx(nc.sbuf_tensor([P, TILES], F32))
        lg = en_ctx(nc.sbuf_tensor([P, TILES], F32))
        partial = en_ctx(nc.sbuf_tensor([P, 1], F32))

        to_sem = en_ctx(nc.semaphore("to_sem"))
        num_sem = en_ctx(nc.semaphore("num_sem"))
        cs = [
            [en_ctx(nc.semaphore(f"cs{s}_{c}")) for c in range(slot_chunks[s])]
            for s in range(NBUF)
        ]
        out_sem = en_ctx(nc.semaphore("out_sem"))
        en_sem = en_ctx(nc.semaphore("en_sem"))
        v_sem = en_ctx(nc.semaphore("v_sem"))
        a_sem = en_ctx(nc.semaphore("a_sem"))
        b_sem = en_ctx(nc.semaphore("b_sem"))

        block = en_ctx(nc.Block())

        _thr = {}

        def chunk_wait_threshold(j, c):
            """Cumulative value of cs[j % NBUF][c] once chunk (j, c) landed."""
            key = (j, c)
            if key not in _thr:
                n = sum(1 for j2 in range(j + 1) if j2 % NBUF == j % NBUF
                        and CHN[j2] > c)
                _thr[key] = 16 * n
            return _thr[key]

        @block.sync
        def _(sync):
            for j, c, lo, hi in chunks:
                if c == 0 and j >= NBUF:
                    # slot reuse: only the scalar engine reads tiles now
                    sync.wait_ge(a_sem, acum[j - NBUF + 1])
                sync.dma_start(
                    out=tb[j % NBUF][:, lo:hi], in_=logits_t[j][:, lo:hi]
                ).then_inc(cs[j % NBUF][c], 16)
            sync.wait_ge(v_sem, V_LG)
            sync.dma_start(out=out[:], in_=partial[:]).then_inc(out_sem, 16)

        @block.gpsimd
        def _(gpsimd):
            gpsimd.dma_start(out=off_sb.ap(), in_=toff[:]).then_inc(to_sem, 16)
            gpsimd.wait_ge(to_sem, 16)
            # gathering DMAs fetch every target logit straight from DRAM; the
            # hardware DGE supports one offset per partition per transfer, so
            # one [128, 1] gather per tile column
            for i in range(TILES):
                gpsimd.indirect_dma_start(
                    out=num.ap()[:, i : i + 1],
                    out_offset=None,
                    in_=logits_flat,
                    in_offset=bass.IndirectOffsetOnAxis(
                        ap=off_sb.ap()[:, i : i + 1], axis=0
                    ),
                ).then_inc(num_sem, 16)

        @block.vector
        def _(vector):
            vector.memset(bias_m[:], M)
            vector.memset(bias_eps[:], EPS).then_inc(b_sem, 1)
            # fold multi-chunk tiles' partial sums into their summ column,
            # progressively as each tile's exps finish
            v = 0
            for j in range(TILES):
                if CHN[j] > 1:
                    x0 = xcol[(j, 0)]
                    vector.wait_ge(a_sem, acum[j + 1])
                    vector.wait_ge(v_sem, v)
                    vector.tensor_reduce(
                        summ[:, j : j + 1],
                        summ_x[:, x0 : x0 + CHN[j]],
                        axis=mybir.AxisListType.X,
                        op=Alu.add,
                    ).then_inc(v_sem, 1)
                    v += 1
            # denom = en * (1 - e^M) + summ
            vector.wait_ge(en_sem, 1)
            vector.wait_ge(a_sem, A_E)
            vector.wait_ge(v_sem, V_FOLD)
            vector.scalar_tensor_tensor(
                out=denom[:],
                in0=en[:],
                scalar=1.0 - EXP_M,
                in1=summ[:],
                op0=Alu.mult,
                op1=Alu.add,
            ).then_inc(v_sem, 1)
            # L = num - ln(denom + eps), accumulated per row
            vector.wait_ge(a_sem, A_LND)
            vector.wait_ge(v_sem, V_DEN)
            vector.wait_ge(num_sem, 16 * TILES)
            vector.scalar_tensor_tensor(
                out=lg[:],
                in0=num[:],
                scalar=1.0,
                in1=lnd[:],
                op0=Alu.mult,
                op1=Alu.subtract,
                accum_out=partial[:],
            ).then_inc(v_sem, 1)

        @block.scalar
        def _(scalar):
            scalar.wait_ge(b_sem, 1)
            k = 0
            for j, c, lo, hi in chunks:
                acc = (
                    summ[:, j : j + 1]
                    if CHN[j] == 1
                    else summ_x[:, xcol[(j, c)] : xcol[(j, c)] + 1]
                )
                scalar.wait_ge(a_sem, k)
                scalar.wait_ge(cs[j % NBUF][c], chunk_wait_threshold(j, c))
                scalar.activation(
                    out=g_act[:, 0 : hi - lo],
                    in_=tb[j % NBUF][:, lo:hi],
                    func=Act.Exp,
                    bias=bias_m[:],
                    scale=1.0,
                    accum_out=acc,
                ).then_inc(a_sem, 1)
                k += 1
                if k == acum[9]:
                    # en = exp(num) computed mid-stream: num is gathered by
                    # ~40us, and this keeps it off the end-of-kernel chain
                    scalar.wait_ge(num_sem, 16 * TILES)
                    scalar.activation(
                        out=en[:], in_=num[:], func=Act.Exp
                    ).then_inc(en_sem, 1)
            scalar.wait_ge(v_sem, V_DEN)
            scalar.activation(
                out=lnd[:], in_=denom[:], func=Act.Ln, bias=bias_eps[:]
            ).then_inc(a_sem, 1)

    return nc


def _get_nc():
    if "nc" not in _CACHE:
        _CACHE["nc"] = _build_nc()
    return _CACHE["nc"]


def kernel(logits, targets):
    global LAST_RESULT
    from concourse.bass_utils import run_bass_kernel_spmd

    logits = np.ascontiguousarray(np.asarray(logits), dtype=np.float32)
    targets = np.asarray(targets).astype(np.int64)
    assert logits.shape == (N_TOTAL, C), logits.shape
    assert targets.shape == (N_TOTAL,), targets.shape

    # tile j, partition p holds shard row j*128 + p; offsets are flat element
    # indices into the core's [ROWS, C] shard for the indirect gather DMA
    rows = np.arange(TILES)[None, :] * P + np.arange(P)[:, None]   # [P, TILES]

    in_maps = []
    for k in range(N_CORES):
        lo, hi = k * ROWS, (k + 1) * ROWS
        shard = logits[lo:hi]
        tg = targets[lo:hi]
        toff = (rows * C + tg[rows]).astype(np.int32)
        in_maps.append({"logits": shard, "toff": np.ascontiguousarray(toff)})

    nc = _get_nc()
    result = run_bass_kernel_spmd(
        nc, in_maps, core_ids=list(range(N_CORES)), trace=PROFILE
    )
    LAST_RESULT = result
    total = np.float64(0.0)
    for r in result.results:
        total += np.float64(r["out"].sum())
    return np.float32(-total / N_TOTAL)
